# revision 19
# baseline (speedup 1.0000x reference)
"""Trainium2 Bass kernel for nn_EnhancedCausalModel.

Computation (see reference): an MLP (288->128->64->32) evaluated for 18
action-variants per (agent, batch, time) token, followed by a KL divergence
between softmax(p_with) and softmax(p_without), averaged over the action dim.

Sharding: agents (N=16) split across 8 cores, 2 agents/core -> 2048 base
tokens/core, each with 18 variants.

Device-side algebra per core (features on partitions, tokens on free dim,
4 base tiles of 512 tokens).  HW finding that shapes this kernel: fp32r
matmuls stream at 2.4 GHz (~305 ns for N=512) only when the moving operand
spans all 128 partitions; any K<128 matmul, and any transition between
K-extents, throttles the PE to 1.2 GHz.  So EVERY matmul here is K=128:
  h_obs = W1[:256].T @ obs.T               (2 matmuls, shared by variants)
  L1 per variant v: pv = W1bpad_ra.T @ inb_block   (stationary zero-padded
     to 128 rows; the other variants' rows multiply zero weights)
     3 engine paths for h1 = relu(pv + h_obs + b1), chosen for balance:
       A: extra eye matmul accumulates h_obs into pv, ACT relu exit
       B: DVE stt (pv+b1)+h_obs, DVE tensor_scalar max (2x mode)
       C: DVE stt, ACT relu
  L2: variants are PAIRED (0,1),(2,3)..,(16,zero): stationaries [W2|0] /
     [0|W2] (M=128) accumulate two variants into one PSUM bank; one ACT
     exit relu(q+b2stack) per pair produces a stacked h2 tile
  L3: stationary [W3;W3] (K=128), one matmul per pair, all pairs
     accumulate into one "without" PSUM bank.  Pair (0,1) uses [0;W3]
     (cf0 side only); the "with" variant's h2 (rows 0:64 of that pair
     tile) is exported and its tiny 64x32 L3 runs on the host in float64
     together with the softmax-KL reduction.
The variant loop is software-pipelined (stage skew) so the PE stream never
waits on an exit op issued immediately before it.
"""

import numpy as np

import concourse.bass as bass
import concourse.mybir as mybir
import concourse.tile as tile
from concourse import bacc
from concourse.bass_utils import run_bass_kernel_spmd

F32 = mybir.dt.float32
F32R = mybir.dt.float32r
AF = mybir.ActivationFunctionType
ALU = mybir.AluOpType

B, T = 16, 64
N_AG, D_OBS, D_ACT = 16, 256, 32
S_CF = 16
H1, H2 = 128, 64
N_CORES = 8
AG_PER = N_AG // N_CORES          # 2 agents per core
TOK = AG_PER * B * T              # 2048 tokens per core
TILE = 512
NT = TOK // TILE                  # 4 base tiles
NS = 9                            # action-variant pair slots per base tile
NPB = (NS + 1) // 2               # pair-blocks in the input block (5)
INW = 2 * TILE + NPB * TILE       # input block cols per base tile (3584)
INV_S1 = 1.0 / (S_CF + 1)         # 1/17

# float32r const block column offsets (all stationaries are K=128 x M cols)
C_W1A0 = 0
C_W1A1 = 128
C_EYE = 256
C_W2LO = 384
C_W2HI = 512
C_W1BP = 640                      # 4 padded W1b stationaries (ra = 0,32,64,96)
C_W3X2 = 1152
C_W3HI = 1184                     # [0; W3] for the (with, cf0) pair
C_TOT = 1216
# fp32 const block: b1 [128], b2 stacked twice [128]
C2_TOT = 2

NV = 18                           # 17 action variants + zero variant
ZV = 17                           # index of the zero variant
# engine-path assignment for v in 0..16 (see module docstring)
PATH_A = frozenset({4, 10})
PATH_C = frozenset({0, 3, 7, 11, 14})
# ACT-exit pairs (by second member); the rest exit on DVE
EXIT_ACT = frozenset({1, 3, 5, 7, 9, 11, 13, 15, ZV})


def build_nc():
    nc = bacc.Bacc("TRN2", target_bir_lowering=False, debug=False,
                   num_devices=N_CORES)

    cblk = nc.dram_tensor("cblk", [128, C_TOT], F32R,
                          kind="ExternalInput").ap()
    cblk2 = nc.dram_tensor("cblk2", [128, C2_TOT], F32,
                           kind="ExternalInput").ap()
    inblk = nc.dram_tensor("inblk", [128, NT * INW], F32R,
                           kind="ExternalInput").ap()
    out_h2w = nc.dram_tensor("out_h2w", [128, NT, TILE], F32,
                             kind="ExternalOutput").ap()
    out_wo = nc.dram_tensor("out_wo", [32, NT, TILE], F32,
                            kind="ExternalOutput").ap()

    with tile.TileContext(nc) as tc:
        with (
            tc.tile_pool(name="const", bufs=1) as cpool,
            tc.tile_pool(name="inp", bufs=2) as ipool,
            tc.tile_pool(name="acts", bufs=4) as apool,
            tc.tile_pool(name="fin", bufs=1) as fpool,
            tc.tile_pool(name="pv", bufs=3, space="PSUM") as vp,
            tc.tile_pool(name="pq", bufs=4, space="PSUM") as qp,
            tc.tile_pool(name="pout", bufs=1, space="PSUM") as pp,
        ):
            cb = cpool.tile([128, C_TOT], F32R)
            # W1a chunk first (h_obs matmuls gate the pipeline start); the
            # rest of the consts interleave with tile 0's input DMAs below
            nc.sync.dma_start(out=cb[:, 0:C_EYE], in_=cblk[:, 0:C_EYE])
            cb2 = cpool.tile([128, C2_TOT], F32)
            nc.sync.dma_start(out=cb2[:], in_=cblk2[:])
            w1a0 = cb[:, C_W1A0:C_W1A0 + 128]
            w1a1 = cb[:, C_W1A1:C_W1A1 + 128]
            eye = cb[:, C_EYE:C_EYE + 128]
            w2lo = cb[:, C_W2LO:C_W2LO + 128]
            w2hi = cb[:, C_W2HI:C_W2HI + 128]
            w3x2 = cb[:, C_W3X2:C_W3X2 + 32]
            w3hi = cb[:, C_W3HI:C_W3HI + 32]
            b1 = cb2[:, 0:1]
            b2s = cb2[:, 1:2]

            lgh = fpool.tile([128, NT, TILE], F32)
            lgw = fpool.tile([32, NT, TILE], F32)
            mm = nc.tensor.matmul

            # variant order within a tile: (slot k, side) pairs; v = index
            VAR = [(k, s) for k in range(NS) for s in range(2 if k < 8 else 1)]

            for i in range(NT):
                inb = ipool.tile([128, INW], F32R, tag="inb")
                nc.sync.dma_start(
                    out=inb[:, 0:TILE],
                    in_=inblk[:, i * INW:i * INW + TILE])
                nc.sync.dma_start(
                    out=inb[:, TILE:2 * TILE],
                    in_=inblk[:, i * INW + TILE:i * INW + 2 * TILE])
                if i == 0:
                    # remaining consts land after tile 0's obs chunks
                    nc.sync.dma_start(out=cb[:, C_W1BP:C_TOT],
                                      in_=cblk[:, C_W1BP:C_TOT])
                nc.sync.dma_start(
                    out=inb[:, 2 * TILE:4 * TILE],
                    in_=inblk[:, i * INW + 2 * TILE:i * INW + 4 * TILE])
                if i == 0:
                    nc.sync.dma_start(out=cb[:, C_EYE:C_W1BP],
                                      in_=cblk[:, C_EYE:C_W1BP])
                nc.sync.dma_start(
                    out=inb[:, 4 * TILE:INW],
                    in_=inblk[:, i * INW + 4 * TILE:(i + 1) * INW])
                obs0 = inb[:, 0:TILE]
                obs1 = inb[:, TILE:2 * TILE]

                wo = pp.tile([32, TILE], F32, tag="wo")

                # h_obs = W1a.T @ obs (K=256 in two chunks)
                ps_h = vp.tile([128, TILE], F32, tag="pv")
                if i == 0:
                    # warm the PE p-state during the obs DMA wait: dummy
                    # matmuls on the already-loaded W1a block into the same
                    # bank the real h_obs matmuls overwrite (start=True)
                    for _ in range(10):
                        mm(ps_h[:, 0:128], w1a0, w1a0, start=True, stop=True,
                           skip_group_check=True)
                mm(ps_h[:], w1a0, obs0, start=True, stop=False)
                mm(ps_h[:], w1a1, obs1, start=False, stop=True)
                h_obs = apool.tile([128, TILE], F32R, tag="h_obs")
                nc.scalar.activation(h_obs[:], ps_h[:], AF.Copy)
                # zero-action variant h1 (shares the h_obs PSUM bank)
                h1z = apool.tile([128, TILE], F32R, tag="h1z")
                nc.scalar.activation(h1z[:], ps_h[:], AF.Relu, bias=b1)

                # per-variant / per-pair state across pipeline stages
                pv_t = [None] * NV
                h1_t = [None] * NV
                q_t = [None] * NV
                h2_t = [None] * NV
                h1_t[ZV] = h1z

                def s0(v):  # L1 matmul(s) -> pv PSUM (K=128 always)
                    k, s = VAR[v]
                    ra = 64 * (k % 2) + 32 * s
                    p_blk = k // 2
                    acols = slice(2 * TILE + p_blk * TILE,
                                  2 * TILE + (p_blk + 1) * TILE)
                    w1bp = cb[:, C_W1BP + (ra // 32) * 128:
                              C_W1BP + (ra // 32 + 1) * 128]
                    pv = vp.tile([128, TILE], F32, tag="pv")
                    if v in PATH_A:
                        mm(pv[:], eye, h_obs[:], start=True, stop=False)
                        mm(pv[:], w1bp, inb[:, acols], start=False, stop=True)
                    else:
                        mm(pv[:], w1bp, inb[:, acols], start=True, stop=True)
                    pv_t[v] = pv

                def s1(v):  # h1 = relu(pre1 + b1), exiting PSUM
                    h1 = apool.tile([128, TILE], F32R, tag="h1")
                    if v in PATH_A:
                        nc.scalar.activation(h1[:], pv_t[v][:], AF.Relu,
                                             bias=b1)
                    else:
                        h1p = apool.tile([128, TILE], F32, tag="h1p")
                        nc.vector.scalar_tensor_tensor(
                            h1p[:], pv_t[v][:], b1, h_obs[:],
                            op0=ALU.add, op1=ALU.add)
                        if v in PATH_C:
                            nc.scalar.activation(h1[:], h1p[:], AF.Relu)
                        else:
                            nc.vector.tensor_scalar(
                                h1[:], h1p[:], 0.0, None, op0=ALU.max)
                    h1_t[v] = h1

                def s2(v):  # L2 matmul into the pair bank (M=128, K=128)
                    if v % 2 == 0 and v != ZV:  # first member of pair
                        q1 = qp.tile([128, TILE], F32, tag="q")
                        mm(q1[:], w2lo, h1_t[v][:], start=True, stop=False)
                        q_t[v] = q1
                    else:                       # second member accumulates
                        q1 = q_t[v - 1] if v != ZV else q_t[16]
                        mm(q1[:], w2hi, h1_t[v][:], start=False, stop=True)
                        q_t[v] = q1

                def s3(v):  # h2 pair exit: relu(q + b2s), full 128 rows
                    q1 = q_t[v]
                    h2 = apool.tile([128, TILE], F32R, tag="h2")
                    if v in EXIT_ACT:
                        nc.scalar.activation(h2[:], q1[:], AF.Relu, bias=b2s)
                    else:
                        nc.vector.tensor_scalar(
                            h2[:], q1[:], b2s, 0.0, op0=ALU.add, op1=ALU.max)
                    h2_t[v] = h2

                def s4(v):  # L3 matmul (K=128); pairs accumulate into wo
                    if v == 1:      # (with, cf0) pair: only the cf0 half
                        mm(wo[:], w3hi, h2_t[v][:], start=True, stop=False,
                           skip_group_check=True)
                        # export the with-variant h2 (rows 0:64 used on host)
                        nc.scalar.activation(lgh[:, i, :], h2_t[v][:], AF.Copy)
                    else:
                        mm(wo[:], w3x2, h2_t[v][:], start=False,
                           stop=(v == ZV), skip_group_check=True)

                # software-pipelined emission.  s2 runs at v+3; pair exits
                # (s3) fire on the second member (odd v / ZV) at +5; L3 (s4)
                # at +6.  The deep skew gives the exit engines (at ~95% of
                # PE load) slack so their jitter never stalls the PE stream.
                for t in range(NV + 7):
                    if t < ZV:
                        s0(t)
                    if 0 <= t - 3 <= ZV:
                        s2(t - 3)
                    v3 = t - 6
                    if 0 <= v3 <= ZV and (v3 % 2 == 1 or v3 == ZV):
                        s3(v3)
                    v4 = t - 7
                    if 0 <= v4 <= ZV and (v4 % 2 == 1 or v4 == ZV):
                        s4(v4)
                    if 0 <= t - 1 < ZV:
                        s1(t - 1)

                # extract this tile's p_without logits (frees the wo bank)
                nc.scalar.activation(lgw[:, i, :], wo[:], AF.Copy)
                nc.sync.dma_start(out=out_h2w[:, i, :], in_=lgh[:, i, :])
                nc.sync.dma_start(out=out_wo[:, i, :], in_=lgw[:, i, :])

    nc.compile()
    return nc


def prep_shared(W1, b1, W2, b2, W3, b3):
    cblk = np.zeros((128, C_TOT), np.float32)
    cblk[:, C_W1A0:C_W1A0 + 128] = W1[0:128]
    cblk[:, C_W1A1:C_W1A1 + 128] = W1[128:256]
    cblk[:, C_EYE:C_EYE + 128] = np.eye(128, dtype=np.float32)
    cblk[0:128, C_W2LO:C_W2LO + 64] = W2
    cblk[0:128, C_W2HI + 64:C_W2HI + 128] = W2
    w1b = W1[D_OBS:]                                   # [32, 128]
    for j in range(4):
        cblk[32 * j:32 * j + 32, C_W1BP + j * 128:C_W1BP + (j + 1) * 128] = w1b
    cblk[0:64, C_W3X2:C_W3X2 + 32] = W3
    cblk[64:128, C_W3X2:C_W3X2 + 32] = W3
    cblk[64:128, C_W3HI:C_W3HI + 32] = W3
    cblk2 = np.zeros((128, C2_TOT), np.float32)
    cblk2[:, 0] = b1
    cblk2[0:64, 1] = b2
    cblk2[64:128, 1] = b2
    return dict(cblk=cblk, cblk2=cblk2)


def prep_core(obs, actions, cf_actions, c):
    n0 = AG_PER * c
    obs_c = obs[:, :, n0:n0 + AG_PER, :]                    # [B,T,2,D]
    obs_t = np.transpose(obs_c, (3, 2, 0, 1)).reshape(D_OBS, TOK)
    act_w = np.transpose(actions[:, :, n0:n0 + AG_PER, :],
                         (3, 2, 0, 1)).reshape(D_ACT, TOK)
    cf_c = cf_actions[n0:n0 + AG_PER]                       # [2,S,B,T,A]
    cf_tok = np.transpose(cf_c, (4, 1, 0, 2, 3)).reshape(D_ACT, S_CF, TOK)
    # slot composition: (with, cf0), (cf1,cf2)..(cf13,cf14), (cf15, -)
    vA = [act_w] + [cf_tok[:, 2 * k - 1] for k in range(1, 8)] + [cf_tok[:, 15]]
    vB = [cf_tok[:, 0]] + [cf_tok[:, 2 * k] for k in range(1, 8)]
    inblk = np.zeros((128, NT * INW), np.float32)
    for i in range(NT):
        tokc = slice(i * TILE, (i + 1) * TILE)
        base = i * INW
        inblk[0:128, base:base + TILE] = obs_t[0:128, tokc]
        inblk[0:128, base + TILE:base + 2 * TILE] = obs_t[128:256, tokc]
        for k in range(NS):
            p_blk, ra = k // 2, 64 * (k % 2)
            blk = slice(base + 2 * TILE + p_blk * TILE,
                        base + 2 * TILE + (p_blk + 1) * TILE)
            inblk[ra:ra + 32, blk] = vA[k][:, tokc]
            if k < 8:
                inblk[ra + 32:ra + 64, blk] = vB[k][:, tokc]
    return dict(inblk=inblk)


def outputs_to_influence(h2w, wo_lg, W3, b3):
    """h2w: [128, NT, 512], wo_lg: [32, NT, 512] -> influence [TOK] (f64)."""
    h2 = h2w.astype(np.float64)[0:64].reshape(64, TOK)
    wo = wo_lg.astype(np.float64).reshape(D_ACT, TOK)
    b3c = b3.astype(np.float64)[:, None]
    w = W3.astype(np.float64).T @ h2 + b3c
    z = wo * INV_S1 + b3c
    e_z = np.exp(z)
    e_w = np.exp(w)
    ez = e_z.sum(axis=0)
    ew = e_w.sum(axis=0)
    s1 = (e_z * (z - w)).sum(axis=0)
    return ((s1 / ez - np.log(ez) + np.log(ew)) / float(D_ACT)).astype(np.float32)


_NC_CACHE = {}


def run_device(inputs, trace=False):
    if "nc" not in _NC_CACHE:
        _NC_CACHE["nc"] = build_nc()
    nc = _NC_CACHE["nc"]
    shared = prep_shared(np.asarray(inputs["W1"]), np.asarray(inputs["b1"]),
                         np.asarray(inputs["W2"]), np.asarray(inputs["b2"]),
                         np.asarray(inputs["W3"]), np.asarray(inputs["b3"]))
    maps = []
    for c in range(N_CORES):
        m = dict(shared)
        m.update(prep_core(np.asarray(inputs["obs"]),
                           np.asarray(inputs["actions"]),
                           np.asarray(inputs["cf_actions"]), c))
        maps.append(m)
    res = run_bass_kernel_spmd(nc, maps, list(range(N_CORES)), trace=trace)
    return res


def kernel(**inputs):
    res = run_device(inputs, trace=False)
    W3 = np.asarray(inputs["W3"])
    b3 = np.asarray(inputs["b3"])
    out = np.empty((B, T, N_AG), np.float32)
    for c in range(N_CORES):
        infl = outputs_to_influence(res.results[c]["out_h2w"],
                                    res.results[c]["out_wo"], W3, b3)
        r = infl.reshape(AG_PER, B, T)
        for a in range(AG_PER):
            out[:, :, AG_PER * c + a] = r[a]
    return out


# revision 20
# speedup vs baseline: 1.1682x; 1.1682x over previous
"""Trainium2 Bass kernel for nn_EnhancedCausalModel.

Computation (see reference): an MLP (288->128->64->32) evaluated for 18
action-variants per (agent, batch, time) token, followed by a KL divergence
between softmax(p_with) and softmax(p_without), averaged over the action dim.

Sharding: agents (N=16) split across 8 cores, 2 agents/core -> 2048 base
tokens/core, each with 18 variants.

Device-side algebra per core (features on partitions, tokens on free dim,
4 base tiles of 512 tokens).  HW finding that shapes this kernel: fp32r
matmuls stream at 2.4 GHz (~305 ns for N=512) only when the moving operand
spans all 128 partitions; any K<128 matmul, and any transition between
K-extents, throttles the PE to 1.2 GHz.  So EVERY matmul here is K=128:
  h_obs = W1[:256].T @ obs.T               (2 matmuls, shared by variants)
  L1 per variant v: pv = W1bpad_ra.T @ inb_block   (stationary zero-padded
     to 128 rows; the other variants' rows multiply zero weights)
     3 engine paths for h1 = relu(pv + h_obs + b1), chosen for balance:
       A: extra eye matmul accumulates h_obs into pv, ACT relu exit
       B: DVE stt (pv+b1)+h_obs, DVE tensor_scalar max (2x mode)
       C: DVE stt, ACT relu
  L2: variants are PAIRED (0,1),(2,3)..,(16,zero): stationaries [W2|0] /
     [0|W2] (M=128) accumulate two variants into one PSUM bank; one ACT
     exit relu(q+b2stack) per pair produces a stacked h2 tile
  L3: stationary [W3;W3] (K=128), one matmul per pair, all pairs
     accumulate into one "without" PSUM bank.  Pair (0,1) uses [0;W3]
     (cf0 side only); the "with" variant's h2 (rows 0:64 of that pair
     tile) is exported and its tiny 64x32 L3 runs on the host in float64
     together with the softmax-KL reduction.
The variant loop is software-pipelined (stage skew) so the PE stream never
waits on an exit op issued immediately before it.
"""

import numpy as np

import concourse.bass as bass
import concourse.mybir as mybir
import concourse.tile as tile
from concourse import bacc
from concourse.bass_utils import run_bass_kernel_spmd

F32 = mybir.dt.float32
F32R = mybir.dt.float32r
AF = mybir.ActivationFunctionType
ALU = mybir.AluOpType

B, T = 16, 64
N_AG, D_OBS, D_ACT = 16, 256, 32
S_CF = 16
H1, H2 = 128, 64
N_CORES = 8
AG_PER = N_AG // N_CORES          # 2 agents per core
TOK = AG_PER * B * T              # 2048 tokens per core
TILE = 512
NT = TOK // TILE                  # 4 base tiles
NS = 9                            # action-variant pair slots per base tile
NPB = (NS + 1) // 2               # pair-blocks in the input block (5)
INW = 2 * TILE + NPB * TILE       # input block cols per base tile (3584)
INV_S1 = 1.0 / (S_CF + 1)         # 1/17

# float32r const block column offsets (all stationaries are K=128 x M cols)
C_W1A0 = 0
C_W1A1 = 128
C_EYE = 256
C_W2LO = 384
C_W2HI = 512
C_W1BP = 640                      # 4 padded W1b stationaries (ra = 0,32,64,96)
C_W3X2 = 1152
C_W3HI = 1184                     # [0; W3] for the (with, cf0) pair
C_TOT = 1216
# fp32 const block: b1 [128], b2 stacked twice [128]
C2_TOT = 2

NV = 18                           # 17 action variants + zero variant
ZV = 17                           # index of the zero variant
# engine-path assignment for v in 0..16 (see module docstring)
PATH_A = frozenset({4, 10})
PATH_C = frozenset({0, 3, 7, 11, 14})
# ACT-exit pairs (by second member); the rest exit on DVE
EXIT_ACT = frozenset({1, 3, 5, 7, 9, 11, 13, 15, ZV})


def build_nc():
    nc = bacc.Bacc("TRN2", target_bir_lowering=False, debug=False,
                   num_devices=N_CORES)

    cblk = nc.dram_tensor("cblk", [128, C_TOT], F32R,
                          kind="ExternalInput").ap()
    cblk2 = nc.dram_tensor("cblk2", [128, C2_TOT], F32,
                           kind="ExternalInput").ap()
    inblk = nc.dram_tensor("inblk", [128, NT * INW], F32R,
                           kind="ExternalInput").ap()
    out_h2w = nc.dram_tensor("out_h2w", [128, NT, TILE], F32,
                             kind="ExternalOutput").ap()
    out_wo = nc.dram_tensor("out_wo", [32, NT, TILE], F32,
                            kind="ExternalOutput").ap()

    with tile.TileContext(nc) as tc:
        with (
            tc.tile_pool(name="const", bufs=1) as cpool,
            tc.tile_pool(name="inp", bufs=2) as ipool,
            tc.tile_pool(name="acts", bufs=4) as apool,
            tc.tile_pool(name="fin", bufs=1) as fpool,
            tc.tile_pool(name="pv", bufs=3, space="PSUM") as vp,
            tc.tile_pool(name="pq", bufs=4, space="PSUM") as qp,
            tc.tile_pool(name="pout", bufs=1, space="PSUM") as pp,
        ):
            cb = cpool.tile([128, C_TOT], F32R)
            # W1a chunk first (h_obs matmuls gate the pipeline start); the
            # rest of the consts interleave with tile 0's input DMAs below
            nc.sync.dma_start(out=cb[:, 0:C_EYE], in_=cblk[:, 0:C_EYE])
            cb2 = cpool.tile([128, C2_TOT], F32)
            nc.sync.dma_start(out=cb2[:], in_=cblk2[:])
            w1a0 = cb[:, C_W1A0:C_W1A0 + 128]
            w1a1 = cb[:, C_W1A1:C_W1A1 + 128]
            eye = cb[:, C_EYE:C_EYE + 128]
            w2lo = cb[:, C_W2LO:C_W2LO + 128]
            w2hi = cb[:, C_W2HI:C_W2HI + 128]
            w3x2 = cb[:, C_W3X2:C_W3X2 + 32]
            w3hi = cb[:, C_W3HI:C_W3HI + 32]
            b1 = cb2[:, 0:1]
            b2s = cb2[:, 1:2]

            lgh = fpool.tile([128, NT, TILE], F32)
            lgw = fpool.tile([32, NT, TILE], F32)
            mm = nc.tensor.matmul

            # variant order within a tile: (slot k, side) pairs; v = index
            VAR = [(k, s) for k in range(NS) for s in range(2 if k < 8 else 1)]

            for i in range(NT):
                inb = ipool.tile([128, INW], F32R, tag="inb")
                nc.sync.dma_start(
                    out=inb[:, 0:TILE],
                    in_=inblk[:, i * INW:i * INW + TILE])
                nc.sync.dma_start(
                    out=inb[:, TILE:2 * TILE],
                    in_=inblk[:, i * INW + TILE:i * INW + 2 * TILE])
                if i == 0:
                    # remaining consts land after tile 0's obs chunks
                    nc.sync.dma_start(out=cb[:, C_W1BP:C_TOT],
                                      in_=cblk[:, C_W1BP:C_TOT])
                nc.sync.dma_start(
                    out=inb[:, 2 * TILE:4 * TILE],
                    in_=inblk[:, i * INW + 2 * TILE:i * INW + 4 * TILE])
                if i == 0:
                    nc.sync.dma_start(out=cb[:, C_EYE:C_W1BP],
                                      in_=cblk[:, C_EYE:C_W1BP])
                nc.sync.dma_start(
                    out=inb[:, 4 * TILE:INW],
                    in_=inblk[:, i * INW + 4 * TILE:(i + 1) * INW])
                obs0 = inb[:, 0:TILE]
                obs1 = inb[:, TILE:2 * TILE]

                wo = pp.tile([32, TILE], F32, tag="wo")

                # h_obs = W1a.T @ obs (K=256 in two chunks)
                ps_h = vp.tile([128, TILE], F32, tag="pv")
                if i == 0:
                    # warm the PE p-state during the obs DMA wait: dummy
                    # matmuls on the already-loaded W1a block into the same
                    # bank the real h_obs matmuls overwrite (start=True)
                    for _ in range(10):
                        mm(ps_h[:, 0:128], w1a0, w1a0, start=True, stop=True,
                           skip_group_check=True)
                mm(ps_h[:], w1a0, obs0, start=True, stop=False)
                mm(ps_h[:], w1a1, obs1, start=False, stop=True)
                h_obs = apool.tile([128, TILE], F32R, tag="h_obs")
                nc.scalar.activation(h_obs[:], ps_h[:], AF.Copy)
                # zero-action variant h1 (shares the h_obs PSUM bank)
                h1z = apool.tile([128, TILE], F32R, tag="h1z")
                nc.scalar.activation(h1z[:], ps_h[:], AF.Relu, bias=b1)

                # per-variant / per-pair state across pipeline stages
                pv_t = [None] * NV
                h1_t = [None] * NV
                q_t = [None] * NV
                h2_t = [None] * NV
                h1_t[ZV] = h1z

                def s0(v):  # L1 matmul(s) -> pv PSUM (K=128 always)
                    k, s = VAR[v]
                    ra = 64 * (k % 2) + 32 * s
                    p_blk = k // 2
                    acols = slice(2 * TILE + p_blk * TILE,
                                  2 * TILE + (p_blk + 1) * TILE)
                    w1bp = cb[:, C_W1BP + (ra // 32) * 128:
                              C_W1BP + (ra // 32 + 1) * 128]
                    pv = vp.tile([128, TILE], F32, tag="pv")
                    if v in PATH_A:
                        mm(pv[:], eye, h_obs[:], start=True, stop=False)
                        mm(pv[:], w1bp, inb[:, acols], start=False, stop=True)
                    else:
                        mm(pv[:], w1bp, inb[:, acols], start=True, stop=True)
                    pv_t[v] = pv

                def s1(v):  # h1 = relu(pre1 + b1), exiting PSUM
                    h1 = apool.tile([128, TILE], F32R, tag="h1")
                    if v in PATH_A:
                        nc.scalar.activation(h1[:], pv_t[v][:], AF.Relu,
                                             bias=b1)
                    else:
                        h1p = apool.tile([128, TILE], F32, tag="h1p")
                        nc.vector.scalar_tensor_tensor(
                            h1p[:], pv_t[v][:], b1, h_obs[:],
                            op0=ALU.add, op1=ALU.add)
                        if v in PATH_C:
                            nc.scalar.activation(h1[:], h1p[:], AF.Relu)
                        else:
                            nc.vector.tensor_scalar(
                                h1[:], h1p[:], 0.0, None, op0=ALU.max)
                    h1_t[v] = h1

                def s2(v):  # L2 matmul into the pair bank (M=128, K=128)
                    if v % 2 == 0 and v != ZV:  # first member of pair
                        q1 = qp.tile([128, TILE], F32, tag="q")
                        mm(q1[:], w2lo, h1_t[v][:], start=True, stop=False)
                        q_t[v] = q1
                    else:                       # second member accumulates
                        q1 = q_t[v - 1] if v != ZV else q_t[16]
                        mm(q1[:], w2hi, h1_t[v][:], start=False, stop=True)
                        q_t[v] = q1

                def s3(v):  # h2 pair exit: relu(q + b2s), full 128 rows
                    q1 = q_t[v]
                    h2 = apool.tile([128, TILE], F32R, tag="h2")
                    if v in EXIT_ACT:
                        nc.scalar.activation(h2[:], q1[:], AF.Relu, bias=b2s)
                    else:
                        nc.vector.tensor_scalar(
                            h2[:], q1[:], b2s, 0.0, op0=ALU.add, op1=ALU.max)
                    h2_t[v] = h2

                def s4(v):  # L3 matmul (K=128); pairs accumulate into wo
                    if v == 1:      # (with, cf0) pair: only the cf0 half
                        mm(wo[:], w3hi, h2_t[v][:], start=True, stop=False,
                           skip_group_check=True)
                        # export the with-variant h2 (rows 0:64 used on host)
                        nc.scalar.activation(lgh[:, i, :], h2_t[v][:], AF.Copy)
                    else:
                        mm(wo[:], w3x2, h2_t[v][:], start=False,
                           stop=(v == ZV), skip_group_check=True)

                # software-pipelined emission.  s2 runs at v+3; pair exits
                # (s3) fire on the second member (odd v / ZV) at +5; L3 (s4)
                # at +6.  The deep skew gives the exit engines (at ~95% of
                # PE load) slack so their jitter never stalls the PE stream.
                for t in range(NV + 6):
                    if t < ZV:
                        s0(t)
                    if 0 <= t - 3 <= ZV:
                        s2(t - 3)
                    v3 = t - 5
                    if 0 <= v3 <= ZV and (v3 % 2 == 1 or v3 == ZV):
                        s3(v3)
                    v4 = t - 6
                    if 0 <= v4 <= ZV and (v4 % 2 == 1 or v4 == ZV):
                        s4(v4)
                    if 0 <= t - 1 < ZV:
                        s1(t - 1)

                # extract this tile's p_without logits (frees the wo bank)
                nc.scalar.activation(lgw[:, i, :], wo[:], AF.Copy)
                nc.sync.dma_start(out=out_h2w[:, i, :], in_=lgh[:, i, :])
                nc.sync.dma_start(out=out_wo[:, i, :], in_=lgw[:, i, :])

    nc.compile()
    return nc


def prep_shared(W1, b1, W2, b2, W3, b3):
    cblk = np.zeros((128, C_TOT), np.float32)
    cblk[:, C_W1A0:C_W1A0 + 128] = W1[0:128]
    cblk[:, C_W1A1:C_W1A1 + 128] = W1[128:256]
    cblk[:, C_EYE:C_EYE + 128] = np.eye(128, dtype=np.float32)
    cblk[0:128, C_W2LO:C_W2LO + 64] = W2
    cblk[0:128, C_W2HI + 64:C_W2HI + 128] = W2
    w1b = W1[D_OBS:]                                   # [32, 128]
    for j in range(4):
        cblk[32 * j:32 * j + 32, C_W1BP + j * 128:C_W1BP + (j + 1) * 128] = w1b
    cblk[0:64, C_W3X2:C_W3X2 + 32] = W3
    cblk[64:128, C_W3X2:C_W3X2 + 32] = W3
    cblk[64:128, C_W3HI:C_W3HI + 32] = W3
    cblk2 = np.zeros((128, C2_TOT), np.float32)
    cblk2[:, 0] = b1
    cblk2[0:64, 1] = b2
    cblk2[64:128, 1] = b2
    return dict(cblk=cblk, cblk2=cblk2)


def prep_core(obs, actions, cf_actions, c):
    n0 = AG_PER * c
    obs_c = obs[:, :, n0:n0 + AG_PER, :]                    # [B,T,2,D]
    obs_t = np.transpose(obs_c, (3, 2, 0, 1)).reshape(D_OBS, TOK)
    act_w = np.transpose(actions[:, :, n0:n0 + AG_PER, :],
                         (3, 2, 0, 1)).reshape(D_ACT, TOK)
    cf_c = cf_actions[n0:n0 + AG_PER]                       # [2,S,B,T,A]
    cf_tok = np.transpose(cf_c, (4, 1, 0, 2, 3)).reshape(D_ACT, S_CF, TOK)
    # slot composition: (with, cf0), (cf1,cf2)..(cf13,cf14), (cf15, -)
    vA = [act_w] + [cf_tok[:, 2 * k - 1] for k in range(1, 8)] + [cf_tok[:, 15]]
    vB = [cf_tok[:, 0]] + [cf_tok[:, 2 * k] for k in range(1, 8)]
    inblk = np.zeros((128, NT * INW), np.float32)
    for i in range(NT):
        tokc = slice(i * TILE, (i + 1) * TILE)
        base = i * INW
        inblk[0:128, base:base + TILE] = obs_t[0:128, tokc]
        inblk[0:128, base + TILE:base + 2 * TILE] = obs_t[128:256, tokc]
        for k in range(NS):
            p_blk, ra = k // 2, 64 * (k % 2)
            blk = slice(base + 2 * TILE + p_blk * TILE,
                        base + 2 * TILE + (p_blk + 1) * TILE)
            inblk[ra:ra + 32, blk] = vA[k][:, tokc]
            if k < 8:
                inblk[ra + 32:ra + 64, blk] = vB[k][:, tokc]
    return dict(inblk=inblk)


def outputs_to_influence(h2w, wo_lg, W3, b3):
    """h2w: [128, NT, 512], wo_lg: [32, NT, 512] -> influence [TOK] (f64)."""
    h2 = h2w.astype(np.float64)[0:64].reshape(64, TOK)
    wo = wo_lg.astype(np.float64).reshape(D_ACT, TOK)
    b3c = b3.astype(np.float64)[:, None]
    w = W3.astype(np.float64).T @ h2 + b3c
    z = wo * INV_S1 + b3c
    e_z = np.exp(z)
    e_w = np.exp(w)
    ez = e_z.sum(axis=0)
    ew = e_w.sum(axis=0)
    s1 = (e_z * (z - w)).sum(axis=0)
    return ((s1 / ez - np.log(ez) + np.log(ew)) / float(D_ACT)).astype(np.float32)


_NC_CACHE = {}


def run_device(inputs, trace=False):
    if "nc" not in _NC_CACHE:
        _NC_CACHE["nc"] = build_nc()
    nc = _NC_CACHE["nc"]
    shared = prep_shared(np.asarray(inputs["W1"]), np.asarray(inputs["b1"]),
                         np.asarray(inputs["W2"]), np.asarray(inputs["b2"]),
                         np.asarray(inputs["W3"]), np.asarray(inputs["b3"]))
    maps = []
    for c in range(N_CORES):
        m = dict(shared)
        m.update(prep_core(np.asarray(inputs["obs"]),
                           np.asarray(inputs["actions"]),
                           np.asarray(inputs["cf_actions"]), c))
        maps.append(m)
    res = run_bass_kernel_spmd(nc, maps, list(range(N_CORES)), trace=trace)
    return res


def kernel(**inputs):
    res = run_device(inputs, trace=False)
    W3 = np.asarray(inputs["W3"])
    b3 = np.asarray(inputs["b3"])
    out = np.empty((B, T, N_AG), np.float32)
    for c in range(N_CORES):
        infl = outputs_to_influence(res.results[c]["out_h2w"],
                                    res.results[c]["out_wo"], W3, b3)
        r = infl.reshape(AG_PER, B, T)
        for a in range(AG_PER):
            out[:, :, AG_PER * c + a] = r[a]
    return out
